# revision 12
# baseline (speedup 1.0000x reference)
"""Trainium2 Bass kernel for nn_LinearEmbedded (moe_routing).

Reference computation:
    w = weight1[region_ix]             # (B, C, D) gather per-region weights
    out = einsum('abc,bcd->abd', x, w) + bias1[region_ix][None]

Sharding: the B axis (128 regions) is split across 8 NeuronCores, 16 per
core; the per-region weight/bias gather happens host-side so each core only
receives the 16 gathered slices it needs.  Per b the device computes
out[b] = x_b @ w_b + bias_b as 4 accumulating K=128 fp16 matmuls plus a
K=1 ones-x-bias matmul into one PSUM bank (fp32 accumulate; measured l2
relative error vs the fp32 reference ~3e-4).  fp16 operands halve the
weight-DMA stream, which is the binding roofline (~38us of ~53us total).

Raw Bass (no TileContext) with hand-rolled semaphores and a minimal tail:

Same math/layout as kernel.py, but hand-rolled semaphores and a minimal
kernel tail: Tile's exit drain + double all-engine barrier + EVSEM
butterfly costs ~10us; here the sync engine proves completion through the
data semaphores, then does one dma_reset + sem_clear (so the NEFF stays
re-executable) and ends.

Engine roles:
    sync   - w half-loads + bias (HWDGE SP ring), final cleanup
    scalar - xt loads + out stores (HWDGE ACT ring), xt prefetched ahead of
             the copy-gated out dispatches so they never head-of-line block
    tensor - 4 accumulating K=128 matmuls + K=1 bias matmul per b
    vector - PSUM -> SBUF copies

DMA rings complete out of order across their parallel queues, so each ring
slot gets its own completion semaphore with at most one outstanding DMA
(slot reuse is already serialized by the consumer-progress semaphores) —
per-slot counting is then exact.

Rings: 6 xt slots, 6 w slots, 4 PSUM banks, 3 out tiles.
"""

import numpy as np

A, B, C, D = 128, 128, 512, 512
NCORES = 8
BL = B // NCORES
KC = C // 128
R_X, R_W, R_P, R_O = 6, 6, 4, 3

_prog = None


def _build_program():
    global _prog
    if _prog is not None:
        return _prog

    import concourse.bass as bass
    import concourse.mybir as mybir
    from contextlib import ExitStack

    F32 = mybir.dt.float32
    F16 = mybir.dt.float16
    nc = bass.Bass("TRN2", target_bir_lowering=False, debug=False)
    xt = nc.dram_tensor("xt", [BL, 128, KC * A], F16, kind="ExternalInput")
    w = nc.dram_tensor("w", [BL, 128, KC * D], F16, kind="ExternalInput")
    bias = nc.dram_tensor("bias", [1, BL * D + A], F16, kind="ExternalInput")
    out = nc.dram_tensor("out", [BL, A, D], F32, kind="ExternalOutput")

    ctx = ExitStack()
    with ctx:
        xts = [
            ctx.enter_context(nc.sbuf_tensor(f"xts{i}", [128, KC * A], F16))
            for i in range(R_X)
        ]
        ws = [
            ctx.enter_context(nc.sbuf_tensor(f"ws{i}", [128, KC * D], F16))
            for i in range(R_W)
        ]
        ots = [
            ctx.enter_context(nc.sbuf_tensor(f"ots{i}", [128, D], F32))
            for i in range(R_O)
        ]
        bias_t = ctx.enter_context(nc.sbuf_tensor("bias_t", [1, BL * D + A], F16))
        psums = [
            ctx.enter_context(nc.psum_tensor(f"psums{i}", [A, D], F32))
            for i in range(R_P)
        ]

        s_xs = [ctx.enter_context(nc.semaphore(f"s_x{i}")) for i in range(R_X)]
        s_wlo = [ctx.enter_context(nc.semaphore(f"s_wlo{i}")) for i in range(R_W)]
        s_whi = [ctx.enter_context(nc.semaphore(f"s_whi{i}")) for i in range(R_W)]
        s_os = [ctx.enter_context(nc.semaphore(f"s_o{i}")) for i in range(R_O)]
        s_w00 = ctx.enter_context(nc.semaphore("s_w00"))
        s_b = ctx.enter_context(nc.semaphore("s_b"))
        s_pe = ctx.enter_context(nc.semaphore("s_pe"))
        s_cp = ctx.enter_context(nc.semaphore("s_cp"))
        s_done = ctx.enter_context(nc.semaphore("s_done"))
        sems = s_xs + s_wlo + s_whi + s_os + [s_w00, s_b, s_pe, s_cp, s_done]

        sync, scalar, tensor, vector = nc.sync, nc.scalar, nc.tensor, nc.vector

        # --- SP engine: w half-loads + bias, then completion proof ---
        if True:
            half = 2 * D
            for b in range(BL):
                if b >= R_W:
                    sync.wait_ge(s_pe, b - R_W + 1)
                if b == 0:
                    # split b=0's low half so the very first matmul only
                    # waits for a 256KB chunk
                    sync.dma_start(ws[0][:, 0:D], w[0, :, 0:D]).then_inc(s_w00, 16)
                    sync.dma_start(ws[0][:, D:half], w[0, :, D:half]).then_inc(
                        s_wlo[0], 16
                    )
                else:
                    sync.dma_start(
                        ws[b % R_W][:, 0:half], w[b, :, 0:half]
                    ).then_inc(s_wlo[b % R_W], 16)
                sync.dma_start(
                    ws[b % R_W][:, half : 2 * half], w[b, :, half : 2 * half]
                ).then_inc(s_whi[b % R_W], 16)
                if b == 0:
                    sync.dma_start(bias_t[:], bias[:]).then_inc(s_b, 16)

            # tail: prove everything landed, then reset for re-execution
            sync.wait_ge(s_pe, BL)
            sync.wait_ge(s_cp, BL)
            sync.wait_ge(s_b, 16)
            for i in range(R_W):
                sync.wait_ge(s_wlo[i], 16 * ((BL - 1 - i) // R_W + 1))
                sync.wait_ge(s_whi[i], 16 * ((BL - 1 - i) // R_W + 1))
            for i in range(R_X):
                sync.wait_ge(s_xs[i], 16 * ((BL - 1 - i) // R_X + 1))
            for i in range(R_O):
                sync.wait_ge(s_os[i], 16 * ((BL - 1 - i) // R_O + 1))
            sync.wait_ge(s_w00, 16)
            sync.wait_ge(s_done, 3)

        # --- PE engine ---
        if True:
            ones = bias_t[:, BL * D : BL * D + A]
            for b in range(BL):
                if b >= R_P:
                    tensor.wait_ge(s_cp, b - R_P + 1)
                tensor.wait_ge(s_xs[b % R_X], 16 * (b // R_X + 1))
                for k in range(KC):
                    if k == 0 and b == 0:
                        tensor.wait_ge(s_w00, 16)
                    elif (k == 1 and b == 0) or (k == 0 and b > 0):
                        tensor.wait_ge(s_wlo[b % R_W], 16 * (b // R_W + 1))
                    elif k == 2:
                        tensor.wait_ge(s_whi[b % R_W], 16 * (b // R_W + 1))
                    nc.tensor.matmul(
                        psums[b % R_P][:],
                        xts[b % R_X][:, k * A : (k + 1) * A],
                        ws[b % R_W][:, k * D : (k + 1) * D],
                        start=(k == 0),
                        stop=False,
                    )
                if b == 0:
                    tensor.wait_ge(s_b, 16)
                nc.tensor.matmul(
                    psums[b % R_P][:],
                    ones,
                    bias_t[:, b * D : (b + 1) * D],
                    start=False,
                    stop=True,
                ).then_inc(s_pe, 1)
            tensor.sem_inc(s_done, 1)

        # --- DVE engine ---
        if True:
            for b in range(BL):
                if b >= R_O:
                    vector.wait_ge(s_os[b % R_O], 16 * ((b - R_O) // R_O + 1))
                vector.wait_ge(s_pe, b + 1)
                nc.vector.tensor_copy(ots[b % R_O][:], psums[b % R_P][:]).then_inc(
                    s_cp, 1
                )
            vector.sem_inc(s_done, 1)

        # --- ACT engine: xt prefetch + out stores ---
        if True:
            PF = 4  # xt prefetch depth (<= R_X)
            for b in range(PF):
                scalar.dma_start(xts[b % R_X][:], xt[b, :, :]).then_inc(
                    s_xs[b % R_X], 16
                )
            for b in range(BL):
                if b + PF < BL:
                    if b + PF >= R_X:
                        scalar.wait_ge(s_pe, b + PF - R_X + 1)
                    scalar.dma_start(
                        xts[(b + PF) % R_X][:], xt[b + PF, :, :]
                    ).then_inc(s_xs[(b + PF) % R_X], 16)
                scalar.wait_ge(s_cp, b + 1)
                scalar.dma_start(out[b, :, :], ots[b % R_O][:]).then_inc(
                    s_os[b % R_O], 16
                )
            scalar.sem_inc(s_done, 1)

        # No Block: engine streams end bare.  Output completion is already
        # proven by the SP waits on the per-slot out semaphores, so no
        # drain/all-engine-barrier tail is needed.

    _prog = nc
    return nc


def _shard_inputs(x, region_ix, weight1, bias1):
    x16 = x.astype(np.float16)
    in_maps = []
    for c in range(NCORES):
        bs = slice(c * BL, (c + 1) * BL)
        rloc = region_ix[bs]
        xs = x16[:, bs, :].transpose(1, 2, 0)  # (BL, C, A)
        xtv = np.ascontiguousarray(
            xs.reshape(BL, KC, 128, A).transpose(0, 2, 1, 3)
        ).reshape(BL, 128, KC * A)
        wg = weight1[rloc].astype(np.float16)  # (BL, C, D)
        wdev = np.ascontiguousarray(
            wg.reshape(BL, KC, 128, D).transpose(0, 2, 1, 3)
        ).reshape(BL, 128, KC * D)
        bg = np.concatenate(
            [bias1[rloc].astype(np.float16).reshape(BL * D), np.ones(A, np.float16)]
        ).reshape(1, BL * D + A)
        in_maps.append({"xt": xtv, "w": wdev, "bias": bg})
    return in_maps


def kernel(x, region_ix, weight1, bias1):
    from concourse.bass_utils import run_bass_kernel_spmd

    x = np.asarray(x, dtype=np.float32)
    region_ix = np.asarray(region_ix).astype(np.int64)
    weight1 = np.asarray(weight1, dtype=np.float32)
    bias1 = np.asarray(bias1, dtype=np.float32)

    nc = _build_program()
    in_maps = _shard_inputs(x, region_ix, weight1, bias1)
    res = run_bass_kernel_spmd(nc, in_maps, core_ids=list(range(NCORES)))

    outv = np.empty((A, B, D), dtype=np.float32)
    for c in range(NCORES):
        outv[:, c * BL : (c + 1) * BL, :] = res.results[c]["out"].transpose(1, 0, 2)
    return outv
